# revision 13
# baseline (speedup 1.0000x reference)
"""Trainium2 Bass kernel v3 for nn_CausalFlowModel (LSTM flow model).

Pure data-parallel over 8 cores; per core bc=256 batch cols, sorted
ascending by h_len and split into two interleaved halves A/B of 128
(two software-pipelined chains that keep ACT/DVE/PE overlapped).

  - HX ring [82, 24*256] fp16: slot(t)=t%24 holds [h_{t-1}(72); x_t(9); 1].
    x loaded 12 steps per DMA; h written in place by the cell update.
  - Gate weights [82, 4*128], gate order (i,f,2g,o), zero-padded to 128
    cols per gate (full-width LDWEIGHTS / FWL-eligible).
  - Per step, per half: 3 chain matmuls (i,f,2g) into gp[128,3,HB] plus
    one o-matmul into the shared gpO[128,2,HB]; ONE sigmoid ACT over
    (i,f,2g); sigma(o) for both halves in a single shared ACT.
  - Cell state stored as e=(c+1)/2 in SA block 3. Tail per half is two
    fused scalar_tensor_tensor ops (saves the 2*sg-1 tensor_scalar):
      P  = ([sg'|e] - 0.5) * [si|sf]  = [si*tanh(g)/2 | sf*c/2]
      e' = (P0 + 0.5) + P1
    tanh(c) = ACT(Tanh, e', scale=2, bias=-1)  (affine folded into ACT);
    h = sigma(o) * tc (DVE) written straight into the next ring slot.
  - Ragged h[l-1]/h[l-2] captures: per-step is_equal masks (GpSimd) +
    copy_predicated (DVE) into H1/H2 over static windows.
  - Live-window shrink: ops cover only columns whose l has not expired
    (static per-step slice bounds from min over cores).
"""

import numpy as np

import concourse.bacc as bacc
import concourse.bass as bass
import concourse.mybir as mybir
import concourse.tile as tile
from concourse.bass_utils import run_bass_kernel_spmd

F32 = mybir.dt.float32
F32R = mybir.dt.float32r
F16 = mybir.dt.float16
AF = mybir.ActivationFunctionType
ALU = mybir.AluOpType

B, T, SD, CD = 2048, 512, 8, 8
H = 72
KG = H + CD + 2          # 82 = h(72) + x(9) + ones(1)
NCORES = 8
BC = B // NCORES         # 256
HB = BC // 2             # 128 cols per half
NSLOT = 24               # ring slots
CHUNK = 12               # steps per x DMA
MPAD = 128               # padded gate-weight cols (FWL)


class Cfg:
    def __init__(self):
        self.mm_dt = F16
        self.stt_tail = True     # fused scalar_tensor_tensor tail (e-reparam)
        self.fwl_pad = True      # pad gate weights to 128 cols


# --------------------------------------------------------------------------- #
# host-side preparation
# --------------------------------------------------------------------------- #

def host_prep(inputs: dict, cfg: Cfg):
    lens = np.asarray(inputs["h_lens"]).astype(np.int64)
    order = np.argsort(lens, kind="stable")

    W_ih = np.asarray(inputs["W_ih"], np.float32)
    W_hh = np.asarray(inputs["W_hh"], np.float32)
    b_g = np.asarray(inputs["b_ih"], np.float32) + np.asarray(inputs["b_hh"], np.float32)
    Wg = np.concatenate([W_hh, W_ih, b_g[:, None]], axis=1)   # [288, 82]
    # torch gate order (i,f,g,o) -> kernel order (i,f,g,o) stays, but g x2
    gi, gf, gg, go = np.split(Wg, 4, axis=0)
    Wg = np.concatenate([gi, gf, 2.0 * gg, go], axis=0)       # (i,f,2g,o)
    mcols = MPAD if cfg.fwl_pad else H
    # [82, 4, mcols] with zero padding per gate
    wg4 = np.zeros((KG, 4, mcols), np.float32)
    for g in range(4):
        wg4[:, g, :H] = Wg[g * H:(g + 1) * H].T
    wg_all = wg4.reshape(KG, 4 * mcols)

    def kchunks(wT, chunk=128):
        return [np.ascontiguousarray(wT[s:s + chunk])
                for s in range(0, wT.shape[0], chunk)]

    we1T = np.ascontiguousarray(np.asarray(inputs["enc_W1"], np.float32).T)
    we2T = np.ascontiguousarray(np.asarray(inputs["enc_W2"], np.float32).T)
    we3T = np.ascontiguousarray(np.asarray(inputs["enc_W3"], np.float32).T)
    wd1T = np.ascontiguousarray(np.asarray(inputs["dec_W1"], np.float32).T)
    wd2T = np.ascontiguousarray(np.asarray(inputs["dec_W2"], np.float32).T)
    wd3T = np.ascontiguousarray(np.asarray(inputs["dec_W3"], np.float32).T)

    def bias_cols(b, p=128):
        ncol = (len(b) + p - 1) // p
        out = np.zeros((p, ncol), np.float32)
        for c in range(ncol):
            seg = b[c * p:(c + 1) * p]
            out[: len(seg), c] = seg
        return out

    shared = {
        "wg_all": wg_all.astype(np.float16),
        "we1T": we1T,
        "we2Tk0": kchunks(we2T)[0], "we2Tk1": kchunks(we2T)[1],
        "we3Tk0": kchunks(we3T)[0], "we3Tk1": kchunks(we3T)[1],
        "wd1T": wd1T,
        "wd2Tk0": kchunks(wd2T)[0], "wd2Tk1": kchunks(wd2T)[1],
        "wd2Tk2": kchunks(wd2T)[2],
        "wd3Tk0": kchunks(wd3T)[0], "wd3Tk1": kchunks(wd3T)[1],
        "wd3Tk2": kchunks(wd3T)[2],
        "be1": bias_cols(np.asarray(inputs["enc_b1"], np.float32)),
        "be2": bias_cols(np.asarray(inputs["enc_b2"], np.float32)),
        "be3": bias_cols(np.asarray(inputs["enc_b3"], np.float32), p=64),
        "bd1": bias_cols(np.asarray(inputs["dec_b1"], np.float32)),
        "bd2": bias_cols(np.asarray(inputs["dec_b2"], np.float32)),
        "bd3": bias_cols(np.asarray(inputs["dec_b3"], np.float32), p=8),
    }

    x = np.asarray(inputs["x"], np.float32)
    rnn = np.asarray(inputs["rnn_input"], np.float32)
    deltas = np.asarray(inputs["deltas"], np.float32)

    in_maps, perms = [], []
    lens_hk = np.zeros((NCORES, 2, HB), np.int64)
    for k in range(NCORES):
        perm = order[np.arange(BC) * NCORES + k]       # sorted ascending
        colperm = np.concatenate([perm[0::2], perm[1::2]])
        perms.append(colperm)
        lk = lens[colperm]                             # [256]
        lens_hk[k, 0] = lk[:HB]
        lens_hk[k, 1] = lk[HB:]
        dsel = deltas[colperm, lk - 1, 0].astype(np.float32)   # [256]
        d1 = np.broadcast_to(dsel, (H, BC)).copy()
        d2 = np.broadcast_to(1.0 - dsel, (H, BC)).copy()
        LENS = np.broadcast_to(lk.astype(np.float32), (H, BC)).copy()
        # XD: [10, T*256], col t*256+j = [rnn[colperm[j], t, :]; 1.0]
        rk = rnn[colperm].transpose(2, 1, 0).reshape(CD + 1, T * BC)
        rk = np.concatenate([rk, np.ones((1, T * BC), np.float32)], axis=0)
        m = dict(shared)
        m.update({
            "xTr": np.ascontiguousarray(x[colperm].T),                 # [8,256] f32
            "xTh": np.ascontiguousarray(x[colperm].T).astype(np.float16),
            "XD": np.ascontiguousarray(rk).astype(np.float16),
            "d1": d1, "d2": d2, "LENS": LENS,
        })
        in_maps.append(m)

    # static live windows / capture windows (shared across cores: min/max)
    tvals = np.arange(T + 3)
    lo = np.zeros((2, T + 3), np.int64)       # live start = min_k #{l <= t}
    wlo = np.full((2, T + 3), HB, np.int64)   # capture window per value v
    whi = np.zeros((2, T + 3), np.int64)
    for h in range(2):
        for k in range(NCORES):
            lk = np.sort(lens_hk[k, h])
            cnt_le = np.searchsorted(lk, tvals, side="right")
            cnt_lt = np.searchsorted(lk, tvals, side="left")
            if k == 0:
                lo[h] = cnt_le
            else:
                lo[h] = np.minimum(lo[h], cnt_le)
            wlo[h] = np.minimum(wlo[h], cnt_lt)
            whi[h] = np.maximum(whi[h], cnt_le)
    meta = {
        "lo": lo, "wlo": wlo, "whi": whi,
        "lmin": int(lens.min()), "lmax": int(lens.max()),
    }
    return in_maps, perms, meta


# --------------------------------------------------------------------------- #
# device kernel
# --------------------------------------------------------------------------- #

def build_nc(cfg: Cfg, meta):
    nc = bacc.Bacc("TRN2", target_bir_lowering=False, debug=False,
                   enable_asserts=False, num_devices=NCORES)
    RD = cfg.mm_dt
    lo_t, wlo_t, whi_t = meta["lo"], meta["wlo"], meta["whi"]
    lmin = meta["lmin"]
    mcols = MPAD if cfg.fwl_pad else H
    MP = mcols  # psum partitions written by each gate matmul

    def din(name, shape, dt=F32):
        return nc.dram_tensor(name, list(shape), dt, kind="ExternalInput").ap()

    ins = {
        "xTr": din("xTr", [SD, BC], F32R),
        "xTh": din("xTh", [SD, BC], RD),
        "XD": din("XD", [CD + 2, T * BC], RD),
        "d1": din("d1", [H, BC]), "d2": din("d2", [H, BC]),
        "LENS": din("LENS", [H, BC]),
        "wg_all": din("wg_all", [KG, 4 * mcols], RD),
        "we1T": din("we1T", [SD, 256], F32R),
        "we2Tk0": din("we2Tk0", [128, 256], F32R), "we2Tk1": din("we2Tk1", [128, 256], F32R),
        "we3Tk0": din("we3Tk0", [128, 64], F32R), "we3Tk1": din("we3Tk1", [128, 64], F32R),
        "wd1T": din("wd1T", [H, 288], F32R),
        "wd2Tk0": din("wd2Tk0", [128, 288], F32R), "wd2Tk1": din("wd2Tk1", [128, 288], F32R),
        "wd2Tk2": din("wd2Tk2", [32, 288], F32R),
        "wd3Tk0": din("wd3Tk0", [128, SD], F32R), "wd3Tk1": din("wd3Tk1", [128, SD], F32R),
        "wd3Tk2": din("wd3Tk2", [32, SD], F32R),
        "be1": din("be1", [128, 2]), "be2": din("be2", [128, 2]),
        "be3": din("be3", [64, 1]),
        "bd1": din("bd1", [128, 3]), "bd2": din("bd2", [128, 3]),
        "bd3": din("bd3", [SD, 1]),
    }
    out_dram = nc.dram_tensor("out", [SD, BC], F32, kind="ExternalOutput").ap()

    with tile.TileContext(nc) as tc:
        with tc.tile_pool(name="const", bufs=1) as cpool, \
             tc.tile_pool(name="work", bufs=3) as wpool, \
             tc.tile_pool(name="psum", bufs=2, space="PSUM") as ppool, \
             tc.tile_pool(name="psum1", bufs=1, space="PSUM") as ppool1:

            sb = {}
            def loadc(names):
                for name in names:
                    ap = ins[name]
                    t_ = cpool.tile(list(ap.shape), ap.dtype, name=f"sb_{name}")
                    nc.sync.dma_start(t_, ap)
                    sb[name] = t_
            # startup-critical only; decoder weights + capture tables stream
            # during the first LSTM steps instead of blocking the encoder
            loadc(["wg_all", "we1T", "we2Tk0", "we2Tk1", "we3Tk0", "we3Tk1",
                   "be1", "be2", "be3"])

            # persistent state
            HX = cpool.tile([KG, NSLOT * BC], RD, name="HX")
            # SA: per half [72, 4, HB]: blocks 0-2 = sigma(i,f,2g); block 3 = e
            SAa = cpool.tile([H, 4, HB], RD, name="SAa")
            SAb = cpool.tile([H, 4, HB], RD, name="SAb")
            H1 = cpool.tile([H, BC], RD, name="H1")
            H2 = cpool.tile([H, BC], RD, name="H2")
            I32 = mybir.dt.int32
            Ma = [cpool.tile([H, HB], I32, name=f"Ma{r}") for r in range(2)]
            Mb = [cpool.tile([H, HB], I32, name=f"Mb{r}") for r in range(2)]
            NEG1 = cpool.tile([H, 1], F32, name="NEG1")
            nc.vector.memset(NEG1, -1.0)
            nc.vector.memset(SAa, 0.0)
            nc.gpsimd.memset(SAb, 0.0)
            nc.vector.memset(SAa[:, 3, :], 0.5)   # e0 = (c0+1)/2
            nc.gpsimd.memset(SAb[:, 3, :], 0.5)
            nc.vector.memset(H1, 0.0)
            nc.vector.memset(H2, 0.0)
            nc.gpsimd.memset(HX.bitcast(mybir.dt.uint16), 0)

            # x chunks 0 and 1 (steps 0..23); row KG-1 gets the ones row
            for b_ in range(2):
                nc.sync.dma_start(
                    HX[H:KG, b_ * CHUNK * BC:(b_ + 1) * CHUNK * BC],
                    ins["XD"][:, b_ * CHUNK * BC:(b_ + 1) * CHUNK * BC])

            def mm(out, lhsT, rhs, start=True, stop=True):
                nc.tensor.matmul(out, lhsT, rhs, start=start, stop=stop)

            # ---- encoder MLP -> h0 into slot 0 ----------------------------
            nc.sync.dma_start(HX[0:SD, 0:BC], ins["xTh"])
            ex = wpool.tile([SD, BC], F32R, name="ex")
            nc.sync.dma_start(ex, ins["xTr"])
            ez1p = ppool1.tile([128, 512], F32, name="ez1p", tag="ps")
            for c in range(2):
                mm(ez1p[:, 256 * c:256 * (c + 1)],
                   sb["we1T"][:, 128 * c:128 * (c + 1)], ex)
            ez1 = wpool.tile([128, 512], F32R, name="ez1")
            for c in range(2):
                nc.scalar.activation(ez1[:, 256 * c:256 * (c + 1)],
                                     ez1p[:, 256 * c:256 * (c + 1)],
                                     AF.Tanh, bias=sb["be1"][:, c:c + 1])
            ez2p = ppool1.tile([128, 512], F32, name="ez2p", tag="ps")
            for c in range(2):
                for k in range(2):
                    mm(ez2p[:, 256 * c:256 * (c + 1)],
                       sb[f"we2Tk{k}"][:, 128 * c:128 * (c + 1)],
                       ez1[:, 256 * k:256 * (k + 1)],
                       start=(k == 0), stop=(k == 1))
            ez2 = wpool.tile([128, 512], F32R, name="ez2")
            for c in range(2):
                nc.scalar.activation(ez2[:, 256 * c:256 * (c + 1)],
                                     ez2p[:, 256 * c:256 * (c + 1)],
                                     AF.Tanh, bias=sb["be2"][:, c:c + 1])
            eh0p = ppool1.tile([64, 256], F32, name="eh0p", tag="ps")
            for k in range(2):
                mm(eh0p, sb[f"we3Tk{k}"], ez2[:, 256 * k:256 * (k + 1)],
                   start=(k == 0), stop=(k == 1))
            eh0 = wpool.tile([64, 256], RD, name="eh0")
            nc.scalar.activation(eh0, eh0p, AF.Identity, bias=sb["be3"][:, 0:1])
            nc.sync.dma_start(HX[SD:H, 0:BC], eh0)
            loadc(["LENS", "d1", "d2", "wd1T", "wd2Tk0", "wd2Tk1", "wd2Tk2",
                   "wd3Tk0", "wd3Tk1", "wd3Tk2", "bd1", "bd2", "bd3"])

            wg = sb["wg_all"]
            def wgate(g):
                return wg[:, g * mcols:(g + 1) * mcols]

            # ---- LSTM over T steps ----------------------------------------
            for t in range(T):
                base = (t % NSLOT) * BC
                nbase = ((t + 1) % NSLOT) * BC
                loA = int(lo_t[0][min(t, T)])
                loB = int(lo_t[1][min(t, T)])
                lvA = HB - loA
                lvB = HB - loB
                if lvA <= 0 and lvB <= 0:
                    break
                if t % CHUNK == 0 and t > 0:
                    b_ = t // CHUNK + 1
                    if b_ * CHUNK < T:
                        c0 = (b_ * CHUNK % NSLOT) * BC
                        n_ = min(CHUNK, T - b_ * CHUNK)
                        nc.sync.dma_start(
                            HX[H:KG, c0:c0 + n_ * BC],
                            ins["XD"][:, b_ * CHUNK * BC:(b_ * CHUNK + n_) * BC])

                # ---- matmuls ------------------------------------------------
                # x-part (K=10, rows 72..81 incl ones) accumulated first via
                # the x_mms issued at the end of the previous step; h-part
                # (K=72) is the only chain-critical matmul work.
                loO = min(loA, loB) if (lvA > 0 and lvB > 0) else max(loA, loB)
                if True:  # BISECT: inline x-part every step
                    gpO = ppool.tile([MP, 2, HB], F32, name="gpO", tag="psO")
                    if lvA > 0:
                        rhsxA = HX[H:KG, base + loA:base + HB]
                        gpA = ppool.tile([MP, 3, HB], F32, name="gpA", tag="psA")
                        for c in range(3):
                            mm(gpA[:, c, loA:HB], wgate(c)[H:KG], rhsxA,
                               start=True, stop=False)
                        mm(gpO[:, 0, loA:HB], wgate(3)[H:KG], rhsxA,
                           start=True, stop=False)
                    if lvB > 0:
                        rhsxB = HX[H:KG, base + HB + loB:base + 2 * HB]
                        gpB = ppool.tile([MP, 3, HB], F32, name="gpB", tag="psB")
                        for c in range(3):
                            mm(gpB[:, c, loB:HB], wgate(c)[H:KG], rhsxB,
                               start=True, stop=False)
                        mm(gpO[:, 1, loB:HB], wgate(3)[H:KG], rhsxB,
                           start=True, stop=False)
                else:
                    gpA, gpB, gpO = gpA_n, gpB_n, gpO_n
                if lvA > 0:
                    rhshA = HX[0:H, base + loA:base + HB]
                    for c in range(3):
                        mm(gpA[:, c, loA:HB], wgate(c)[0:H], rhshA,
                           start=False, stop=True)
                    mm(gpO[:, 0, loA:HB], wgate(3)[0:H], rhshA,
                       start=False, stop=True)
                if lvB > 0:
                    rhshB = HX[0:H, base + HB + loB:base + 2 * HB]
                    for c in range(3):
                        mm(gpB[:, c, loB:HB], wgate(c)[0:H], rhshB,
                           start=False, stop=True)
                    mm(gpO[:, 1, loB:HB], wgate(3)[0:H], rhshB,
                       start=False, stop=True)

                # prefetch next step's x-part into fresh psum buffers; fills
                # the PE idle gap (keeps HAM warm) and is off the h chain
                t1 = t + 1
                loA1 = int(lo_t[0][min(t1, T)])
                loB1 = int(lo_t[1][min(t1, T)])
                if False and t1 < T and (loA1 < HB or loB1 < HB):  # BISECT
                    nb1 = (t1 % NSLOT) * BC
                    gpA_n = ppool.tile([MP, 3, HB], F32, name="gpA", tag="psA")
                    gpB_n = ppool.tile([MP, 3, HB], F32, name="gpB", tag="psB")
                    gpO_n = ppool.tile([MP, 2, HB], F32, name="gpO", tag="psO")
                    if loA1 < HB:
                        rhsxA = HX[H:KG, nb1 + loA1:nb1 + HB]
                        for c in range(3):
                            mm(gpA_n[:, c, loA1:HB], wgate(c)[H:KG], rhsxA,
                               start=True, stop=False)
                        mm(gpO_n[:, 0, loA1:HB], wgate(3)[H:KG], rhsxA,
                           start=True, stop=False)
                    if loB1 < HB:
                        rhsxB = HX[H:KG, nb1 + HB + loB1:nb1 + 2 * HB]
                        for c in range(3):
                            mm(gpB_n[:, c, loB1:HB], wgate(c)[H:KG], rhsxB,
                               start=True, stop=False)
                        mm(gpO_n[:, 1, loB1:HB], wgate(3)[H:KG], rhsxB,
                           start=True, stop=False)

                # ---- sigma(i,f,2g) -> SA blocks 0-2 -----------------------
                if lvA > 0:
                    nc.scalar.activation(SAa[:, 0:3, loA:HB], gpA[0:H, 0:3, loA:HB],
                                         AF.Sigmoid)
                if lvB > 0:
                    nc.scalar.activation(SAb[:, 0:3, loB:HB], gpB[0:H, 0:3, loB:HB],
                                         AF.Sigmoid)

                # ---- fused tail: P then e' (DVE stt) ----------------------
                # P = ([sg'|e] - 0.5) * [si|sf]  = [si*tg/2 | sf*c/2]
                if lvA > 0:
                    Pa = wpool.tile([H, 2, HB], RD, name="Pa")
                    nc.vector.scalar_tensor_tensor(
                        Pa[:, :, loA:HB], SAa[:, 2:4, loA:HB], -0.5,
                        SAa[:, 0:2, loA:HB], op0=ALU.add, op1=ALU.mult)
                    nc.vector.scalar_tensor_tensor(
                        SAa[:, 3, loA:HB], Pa[:, 0, loA:HB], 0.5,
                        Pa[:, 1, loA:HB], op0=ALU.add, op1=ALU.add)
                if lvB > 0:
                    Pb = wpool.tile([H, 2, HB], RD, name="Pb")
                    nc.vector.scalar_tensor_tensor(
                        Pb[:, :, loB:HB], SAb[:, 2:4, loB:HB], -0.5,
                        SAb[:, 0:2, loB:HB], op0=ALU.add, op1=ALU.mult)
                    nc.vector.scalar_tensor_tensor(
                        SAb[:, 3, loB:HB], Pb[:, 0, loB:HB], 0.5,
                        Pb[:, 1, loB:HB], op0=ALU.add, op1=ALU.add)

                # ---- sigma(o) shared, tanh per half, h = so*tc ------------
                SO = wpool.tile([H, 2, HB], RD, name="SO")
                if lvA > 0:
                    TCa = wpool.tile([H, HB], RD, name="TCa")
                    nc.scalar.activation(TCa[:, loA:HB], SAa[:, 3, loA:HB],
                                         AF.Tanh, scale=2.0, bias=NEG1[:, 0:1])
                if lvA > 0 and lvB > 0:
                    nc.scalar.activation(SO[:, :, loO:HB], gpO[0:H, :, loO:HB],
                                         AF.Sigmoid)
                elif lvA > 0:
                    nc.scalar.activation(SO[:, 0, loA:HB], gpO[0:H, 0, loA:HB],
                                         AF.Sigmoid)
                else:
                    nc.scalar.activation(SO[:, 1, loB:HB], gpO[0:H, 1, loB:HB],
                                         AF.Sigmoid)
                if lvB > 0:
                    TCb = wpool.tile([H, HB], RD, name="TCb")
                    nc.scalar.activation(TCb[:, loB:HB], SAb[:, 3, loB:HB],
                                         AF.Tanh, scale=2.0, bias=NEG1[:, 0:1])
                if lvA > 0:
                    nc.vector.tensor_tensor(HX[0:H, nbase + loA:nbase + HB],
                                            SO[:, 0, loA:HB], TCa[:, loA:HB],
                                            op=ALU.mult)
                if lvB > 0:
                    nc.vector.tensor_tensor(HX[0:H, nbase + HB + loB:nbase + 2 * HB],
                                            SO[:, 1, loB:HB], TCb[:, loB:HB],
                                            op=ALU.mult)

                # ---- captures (masks + predicated copies) -----------------
                if t >= lmin - 2:
                    for h_, (M, off) in enumerate([(Ma, 0), (Mb, HB)]):
                        v2 = t + 2
                        if v2 <= T and whi_t[h_][v2] > wlo_t[h_][v2]:
                            a, b2 = int(wlo_t[h_][v2]), int(whi_t[h_][v2])
                            eng = nc.gpsimd
                            eng.tensor_scalar(M[t % 2][:, a:b2],
                                              sb["LENS"][:, off + a:off + b2],
                                              float(v2), None, op0=ALU.is_equal)
                            nc.vector.copy_predicated(
                                H2[:, off + a:off + b2], M[t % 2][:, a:b2],
                                HX[0:H, nbase + off + a:nbase + off + b2])
                        v1 = t + 1
                        if v1 >= lmin and whi_t[h_][v1] > wlo_t[h_][v1]:
                            a, b1 = int(wlo_t[h_][v1]), int(whi_t[h_][v1])
                            nc.vector.copy_predicated(
                                H1[:, off + a:off + b1], M[(t + 1) % 2][:, a:b1],
                                HX[0:H, nbase + off + a:nbase + off + b1])

            # ---- dec_in = d1*H1 + d2*H2 -----------------------------------
            U1 = wpool.tile([H, BC], F32, name="U1")
            nc.vector.tensor_tensor(U1, sb["d1"], H1, op=ALU.mult)
            U2 = wpool.tile([H, BC], F32, name="U2")
            nc.vector.tensor_tensor(U2, sb["d2"], H2, op=ALU.mult)
            DI = wpool.tile([H, BC], F32R, name="DI")
            nc.vector.tensor_tensor(DI, U1, U2, op=ALU.add)

            # ---- decoder MLP ----------------------------------------------
            CH1 = [(0, 128), (128, 128), (256, 32)]
            dz1p = ppool1.tile([128, 768], F32, name="dz1p", tag="ps")
            for c, (off, m_) in enumerate(CH1):
                mm(dz1p[0:m_, 256 * c:256 * c + BC], sb["wd1T"][:, off:off + m_], DI)
            dz1 = wpool.tile([128, 768], F32R, name="dz1")
            for c, (off, m_) in enumerate(CH1):
                nc.scalar.activation(dz1[0:m_, 256 * c:256 * c + BC],
                                     dz1p[0:m_, 256 * c:256 * c + BC],
                                     AF.Tanh, bias=sb["bd1"][0:m_, c:c + 1])
            dz2p = ppool1.tile([128, 768], F32, name="dz2p", tag="ps")
            for c, (off, m_) in enumerate(CH1):
                for k, (koff, km) in enumerate(CH1):
                    mm(dz2p[0:m_, 256 * c:256 * c + BC],
                       sb[f"wd2Tk{k}"][0:km, off:off + m_],
                       dz1[0:km, 256 * k:256 * k + BC],
                       start=(k == 0), stop=(k == 2))
            dz2 = wpool.tile([128, 768], F32R, name="dz2")
            for c, (off, m_) in enumerate(CH1):
                nc.scalar.activation(dz2[0:m_, 256 * c:256 * c + BC],
                                     dz2p[0:m_, 256 * c:256 * c + BC],
                                     AF.Tanh, bias=sb["bd2"][0:m_, c:c + 1])
            dz3p = ppool1.tile([SD, 256], F32, name="dz3p", tag="ps")
            for k, (koff, km) in enumerate(CH1):
                mm(dz3p, sb[f"wd3Tk{k}"][0:km, :],
                   dz2[0:km, 256 * k:256 * k + BC],
                   start=(k == 0), stop=(k == 2))
            OUT = wpool.tile([SD, BC], F32, name="OUT")
            nc.scalar.activation(OUT, dz3p, AF.Identity, bias=sb["bd3"][:, 0:1])
            nc.sync.dma_start(out_dram, OUT)

            import os as _os
            if _os.environ.get("KDBG"):
                h1d = nc.dram_tensor("dbg_H1", [H, BC], RD, kind="ExternalOutput").ap()
                h2d = nc.dram_tensor("dbg_H2", [H, BC], RD, kind="ExternalOutput").ap()
                hxd = nc.dram_tensor("dbg_HX", [KG, NSLOT * BC], RD, kind="ExternalOutput").ap()
                did = nc.dram_tensor("dbg_DI", [H, BC], F32, kind="ExternalOutput").ap()
                nc.sync.dma_start(h1d, H1)
                nc.sync.dma_start(h2d, H2)
                nc.sync.dma_start(hxd, HX)
                nc.sync.dma_start(did, DI.bitcast(F32))

    nc.compile()
    return nc


# --------------------------------------------------------------------------- #
# entry point
# --------------------------------------------------------------------------- #

def kernel(**inputs) -> np.ndarray:
    cfg = Cfg()
    in_maps, perms, meta = host_prep(inputs, cfg)
    nc = build_nc(cfg, meta)
    res = run_bass_kernel_spmd(nc, in_maps, core_ids=list(range(NCORES)))
    out = np.empty((B, SD), np.float32)
    for k in range(NCORES):
        out[perms[k]] = res.results[k]["out"].T
    return out
